# revision 1
# baseline (speedup 1.0000x reference)
"""CARAFE (content-aware upsample) Trainium2 kernel.

Sharding: 8 cores = batch(4) x H-halves(2). Host slices X with 2-row
zero-padded halos; each core computes its full output shard
[64, 128, 256]; host concatenates. No collectives.

Key algebraic simplification: dilation (2) == scale (2), so the
unfold patch for hi-res pixel (2h+r1, 2w+r2), tap (i,j) is
X[c, h+i-2, w+j-2] -- independent of the subpixel (r1,r2). The whole
CARAFE accumulation runs on the low-res grid:
  out_q[c,h,w] = sum_ij A[(i*5+j)*4+q, h, w] * X[c, h+i-2, w+j-2]
with A = softmax(pixel-shuffled encoder logits).

Softmax normalization is deferred: E = exp(logits) (logits are O(1),
max-subtraction unnecessary), denominators D_q = sum_k E[4k+q] via a
one-hot matmul, and E^T is scaled by 1/D once in pixel-major layout.
"""

import numpy as np

SCALE = 2
KUP = 5
EPS = 1e-5
B, C, H, W = 4, 64, 128, 128
CMID = 64
ENC = 100  # (SCALE*KUP)**2
HALF = H // 2          # 64 low-res rows per core
HL = HALF + 4          # 68 rows of X incl. 2-row halos
WM1R = HALF + 2        # 66 rows of compressed features (1-row halo)
WM1W = W + 2           # 130 cols (1-col zero pad each side)

USE_F32R = False       # fp32 data streamed at 1 cyc/row on PE (N>=256)
GP_TAPS = 0            # GpSimd lacks TensorScalarPtr in the V3 ISA; keep 0
FUSED_NORM = True      # normalize E^T by 1/D in one big rank-4-AP op
REPS = 1               # in-NEFF repetitions (timing only; leave 1 for grading)


def _build_program():
    import concourse.bass as bass
    import concourse.tile as tile
    from concourse import mybir
    from concourse.vector_clock import ScopedClock

    f32 = mybir.dt.float32
    f32r = mybir.dt.float32r
    mm_dt = f32r if USE_F32R else f32

    class SplitDrainTC(tile.TileContext):
        # walrus in this container rejects >2 sync waits on one CTRL
        # instruction; put each tail-drain wait on its own SP nop.
        def _drain_and_barrier(self, tick_clock, wait_clock):
            probe = self.nc.sync.nop()
            wait_clock.add_sem_waits(
                probe.ins, ScopedClock({None: tick_clock.global_clock})
            )
            waits = list(probe.ins.sync_info.on_wait) if probe.ins.sync_info else []
            if probe.ins.sync_info:
                probe.ins.sync_info.on_wait = []
            for w in waits:
                n = self.nc.sync.nop()
                if n.ins.sync_info is None:
                    n.ins.sync_info = mybir.SyncInfo(on_wait=[w], on_update=[])
                else:
                    n.ins.sync_info.on_wait = [w]
            self.nc.sync.drain()
            self.nc.all_engine_barrier()
            assert self.sems is not None
            popped = self.nc._tile_sem_poison_stack.pop()
            assert popped is self._sem_poison
            self.nc.clear_and_free_semaphores(list(self.sems.allocated().values()))
            self.nc.all_engine_barrier()

    nc = bass.Bass()
    ap_in = {}
    for name, shape in [
        ("Xh", [C, HL * W]),
        ("W1", [C, CMID]),
        ("W3", [C, 9 * ENC]),
        ("c1s", [CMID, 1]),
        ("c1b", [CMID, 1]),
        ("c3s", [ENC, 1]),
        ("c3b", [ENC, 1]),
        ("sel", [ENC, 4]),
        ("ident", [128, 128]),
    ]:
        ap_in[name] = nc.dram_tensor(name, shape, f32, kind="ExternalInput").ap()
    out_d = nc.dram_tensor("out", [C, SCALE * HALF, SCALE * W], f32,
                           kind="ExternalOutput").ap()

    mult = mybir.AluOpType.mult
    add = mybir.AluOpType.add
    AF = mybir.ActivationFunctionType

    with SplitDrainTC(nc) as tc:
        for _ in range(REPS):
            _build_tile_kernel(tc, nc, ap_in, out_d, mm_dt, mult, add, AF,
                               bass, mybir)
    _split_sync_waits(nc, mybir)
    return nc


def _split_sync_waits(nc, mybir, max_waits=1):
    """walrus in this container rejects multiple sync waits on some
    instruction structs (Matmult allows just one);
    hoist the excess onto same-engine nops placed just before."""
    ctr = 0
    for bb in nc.m.functions[0].blocks:
        new = []
        changed = False
        for inst in bb.instructions:
            si = inst.sync_info
            waits = list(si.on_wait) if si and si.on_wait else []
            if len(waits) > max_waits:
                extra, keep = waits[:-max_waits], waits[-max_waits:]
                for i in range(0, len(extra), max_waits):
                    ctr += 1
                    nop = mybir.InstNoOp(name=f"wsplit-{ctr}", ins=[], outs=[])
                    nop.engine = inst.engine
                    nop.sync_info = mybir.SyncInfo(
                        on_wait=extra[i : i + max_waits], on_update=[]
                    )
                    new.append(nop)
                si.on_wait = keep
                changed = True
            new.append(inst)
        if changed:
            bb.instructions = new


def _build_tile_kernel(tc, nc, ap_in, out_d, mm_dt, mult, add, AF, bass, mybir):
    f32 = mybir.dt.float32
    ctxs = []

    def pool(name, bufs, space="SBUF"):
        p = tc.tile_pool(name=name, bufs=bufs, space=space)
        ctxs.append(p)
        return p.__enter__()

    consts = pool("consts", 1)
    persist = pool("persist", 1)
    psA = pool("psA", 2, space="PSUM")     # conv matmul outputs
    psT = pool("psT", 2, space="PSUM")     # transposes
    psD = pool("psD", 1, space="PSUM")     # softmax denominators
    accp = pool("acc", 4)
    stagep = pool("stage", 4)

    # ---- constants ----
    W1 = consts.tile([C, CMID], f32, tag="w1")
    nc.sync.dma_start(W1[:], ap_in["W1"][:])
    W3 = consts.tile([C, 9 * ENC], f32, tag="w3")
    nc.sync.dma_start(W3[:], ap_in["W3"][:])
    c1s = consts.tile([CMID, 1], f32, tag="c1s")
    nc.sync.dma_start(c1s[:], ap_in["c1s"][:])
    c1b = consts.tile([CMID, 1], f32, tag="c1b")
    nc.sync.dma_start(c1b[:], ap_in["c1b"][:])
    c3s = consts.tile([ENC, 1], f32, tag="c3s")
    nc.sync.dma_start(c3s[:], ap_in["c3s"][:])
    c3b = consts.tile([ENC, 1], f32, tag="c3b")
    nc.sync.dma_start(c3b[:], ap_in["c3b"][:])
    sel = consts.tile([ENC, 4], f32, tag="sel")
    nc.sync.dma_start(sel[:], ap_in["sel"][:])
    ident = consts.tile([128, 128], f32, tag="ident")
    nc.sync.dma_start(ident[:], ap_in["ident"][:])

    # ---- load X ----
    Xh = persist.tile([C, HL * W], f32, tag="xh")
    nc.sync.dma_start(Xh[:], ap_in["Xh"][:])

    # ---- compress: 1x1 conv + BN + ReLU -> Wm1 [C, 66 x 130] ----
    Wm1 = persist.tile([C, WM1R * WM1W], f32, tag="wm1")
    nc.gpsimd.memset(Wm1.rearrange("p (r w) -> p r w", w=WM1W)[:, :, 0:1], 0.0)
    nc.gpsimd.memset(
        Wm1.rearrange("p (r w) -> p r w", w=WM1W)[:, :, WM1W - 1 : WM1W], 0.0
    )
    r = 0
    while r < WM1R:
        rows = min(4, WM1R - r)
        n = rows * W
        ps = psA.tile([CMID, 512], f32, tag="ps")
        nc.tensor.matmul(
            ps[:, :n],
            W1[:].bitcast(mm_dt),
            Xh[:, (r + 1) * W : (r + 1 + rows) * W].bitcast(mm_dt),
            start=True, stop=True,
        )
        wm_view = Wm1.rearrange("p (r w) -> p r w", w=WM1W)
        nc.scalar.activation(
            wm_view[:, r : r + rows, 1 : 1 + W],
            ps[:, :n].rearrange("p (r w) -> p r w", w=W),
            AF.Relu, bias=c1b[:], scale=c1s[:],
        )
        r += rows

    # ---- X transpose: Xt [128w, (5 j-shifts, 68 rows, 64 c)] ----
    Xt = persist.tile([128, 5 * HL * C], f32, tag="xt")
    xt_v = Xt.rearrange("p (j r c) -> p j r c", j=5, c=C)
    xh_v = Xh.rearrange("p (r w) -> p r w", w=W)
    for rho in range(HL):
        pt = psT.tile([128, 128], f32, tag="ps")
        nc.tensor.transpose(pt[:, :C], xh_v[:, rho, :], ident[:C, :C])
        nc.scalar.copy(xt_v[:, 2, rho, :], pt[:, :C])
    for j in [0, 1, 3, 4]:
        d = j - 2
        p0, p1 = max(0, -d), 128 - max(0, d)
        # zero the whole band first (gpsimd memset needs aligned start
        # partitions); the shift DMA then overwrites the interior.
        nc.gpsimd.memset(xt_v[:, j, :, :], 0.0)
        nc.sync.dma_start(
            out=xt_v[p0:p1, j, :, :], in_=xt_v[p0 + d : p1 + d, 2, :, :]
        )

    # ---- encode: 3x3 conv + affine + exp -> E [100, 64*128] ----
    # E reuses Xh's SBUF slot (Xh is fully consumed by conv1 + transposes).
    E = persist.tile([ENC, HALF * W], f32, tag="xh")
    wm_flat = Wm1[:]
    for t0 in range(0, HALF, 4):
        ps = psA.tile([ENC, 512], f32, tag="ps")
        for ti, (di, dj) in enumerate((di, dj) for di in range(3) for dj in range(3)):
            off = (t0 + di) * WM1W + dj
            mv = bass.AP(
                tensor=wm_flat.tensor,
                offset=wm_flat.offset + off,
                ap=[wm_flat.ap[0], [WM1W, 4], [1, W]],
            )
            nc.tensor.matmul(
                ps[:],
                W3[:, ti * ENC : (ti + 1) * ENC].bitcast(mm_dt),
                mv.bitcast(mm_dt),
                start=(ti == 0), stop=(ti == 8),
            )
        nc.scalar.activation(
            E[:, t0 * W : (t0 + 4) * W], ps[:], AF.Exp, bias=c3b[:], scale=c3s[:]
        )

    # ---- softmax denominators: D^T then 1/D ----
    pd = psD.tile([128, 4 * HALF], f32, tag="pd")
    for t in range(HALF):
        nc.tensor.matmul(
            pd[:, t * 4 : (t + 1) * 4],
            E[:, t * W : (t + 1) * W].bitcast(mm_dt),
            sel[:].bitcast(mm_dt),
            start=True, stop=True,
        )
    Rt = persist.tile([128, 4 * HALF], f32, tag="rt")
    nc.vector.reciprocal(Rt[:], pd[:])

    # ---- E^T (pixel-major) and fused normalization ----
    # Et reuses Wm1's slot (Wm1 fully consumed by the 3x3 conv).
    Et = persist.tile([128, HALF * ENC], f32, tag="wm1")
    et_v = Et.rearrange("p (t k) -> p t k", k=ENC)
    for t in range(HALF):
        pt = psT.tile([128, 128], f32, tag="ps")
        nc.tensor.transpose(pt[:, :ENC], E[:, t * W : (t + 1) * W], ident[:ENC, :ENC])
        nc.scalar.copy(et_v[:, t, :], pt[:, :ENC])
    if FUSED_NORM:
        bcast = bass.AP(
            tensor=Rt.tensor, offset=Rt.offset,
            ap=[Rt.ap[0], [4, HALF], [0, 25], [1, 4]],
        )
        nc.vector.tensor_tensor(Et[:], Et[:], bcast, mult)
    else:
        for t in range(HALF):
            bc = bass.AP(
                tensor=Rt.tensor, offset=Rt.offset + t * 4,
                ap=[Rt.ap[0], [0, 25], [1, 4]],
            )
            nc.vector.tensor_tensor(et_v[:, t, :], et_v[:, t, :], bc, mult)

    # ---- CARAFE accumulation + pixel-shuffle writeout ----
    taps = [(i, j) for i in range(5) for j in range(5)]
    ndve = len(taps) - GP_TAPS
    for t in range(HALF):
        stage = stagep.tile([C, 2 * 2 * W], f32, tag="stage")
        for q in range(4):
            r1, r2 = q // 2, q % 2
            acc = accp.tile([128, C], f32, tag="acc")
            accg = (accp.tile([128, C], f32, tag="accg", name="accg")
                    if GP_TAPS else None)
            for ki, (i, j) in enumerate(taps):
                ch = (i * 5 + j) * 4 + q
                sc = et_v[:, t, ch : ch + 1]
                x_in = xt_v[:, j, t + i, :]
                if ki < ndve:
                    dst, eng, first = acc, nc.vector, ki == 0
                else:
                    dst, eng, first = accg, nc.gpsimd, ki == ndve
                if first:
                    eng.tensor_scalar(dst[:], x_in, sc, None, mult)
                else:
                    eng.scalar_tensor_tensor(dst[:], x_in, sc, dst[:], mult, add)
            if GP_TAPS:
                nc.vector.tensor_tensor(acc[:], acc[:], accg[:], add)
            po = psT.tile([128, 128], f32, tag="ps")
            nc.tensor.transpose(po[:C, :], acc[:], ident[:, :])
            st_v = stage.rearrange("p (r x) -> p r x", r=2)
            out_ap = bass.AP(
                tensor=st_v.tensor, offset=st_v.offset + r1 * 2 * W + r2,
                ap=[st_v.ap[0], [2, W]],
            )
            nc.scalar.copy(out_ap, po[:C, :])
        nc.sync.dma_start(
            out_d[:, 2 * t : 2 * t + 2, :], stage.rearrange("p (r x) -> p r x", r=2)
        )

    for p in reversed(ctxs):
        p.__exit__(None, None, None)


def _host_inputs(X, comp_w, comp_gamma, comp_beta, comp_mean, comp_var,
                 enc_w, enc_b, enc_gamma, enc_beta, enc_mean, enc_var):
    X = np.asarray(X, np.float32)
    inv1 = (np.asarray(comp_gamma, np.float32)
            / np.sqrt(np.asarray(comp_var, np.float32) + EPS))
    b1 = np.asarray(comp_beta, np.float32) - np.asarray(comp_mean, np.float32) * inv1
    inv3 = (np.asarray(enc_gamma, np.float32)
            / np.sqrt(np.asarray(enc_var, np.float32) + EPS))
    b3 = ((np.asarray(enc_b, np.float32) - np.asarray(enc_mean, np.float32)) * inv3
          + np.asarray(enc_beta, np.float32))

    W1 = np.ascontiguousarray(np.asarray(comp_w, np.float32)[:, :, 0, 0].T)
    # W3[c_in, tap*100 + c_out]
    W3 = np.ascontiguousarray(
        np.asarray(enc_w, np.float32).transpose(2, 3, 1, 0).reshape(9 * C, ENC)
        .reshape(9, C, ENC).transpose(1, 0, 2).reshape(C, 9 * ENC)
    )
    sel = np.zeros((ENC, 4), np.float32)
    sel[np.arange(ENC), np.arange(ENC) % 4] = 1.0
    ident = np.eye(128, dtype=np.float32)

    common = dict(
        W1=W1, W3=W3,
        c1s=inv1.reshape(CMID, 1), c1b=b1.reshape(CMID, 1),
        c3s=inv3.reshape(ENC, 1), c3b=b3.reshape(ENC, 1),
        sel=sel, ident=ident,
    )
    in_maps = []
    for s in range(8):
        b, half = divmod(s, 2)
        h0 = half * HALF
        xs = np.zeros((C, HL, W), np.float32)
        lo, hi = h0 - 2, h0 + HALF + 2
        clo, chi = max(lo, 0), min(hi, H)
        xs[:, clo - lo : clo - lo + (chi - clo), :] = X[b, :, clo:chi, :]
        in_maps.append(dict(Xh=xs.reshape(C, HL * W), **common))
    return in_maps


_PROGRAM_CACHE = {}


def _run(in_maps, trace=False, **kw):
    from concourse.bass_utils import run_bass_kernel_spmd

    if "nc" not in _PROGRAM_CACHE:
        _PROGRAM_CACHE["nc"] = _build_program()
    nc = _PROGRAM_CACHE["nc"]
    return run_bass_kernel_spmd(nc, in_maps, list(range(8)), trace=trace, **kw)


def _gather(res):
    out = np.zeros((B, C, SCALE * H, SCALE * W), np.float32)
    for s in range(8):
        b, half = divmod(s, 2)
        out[b, :, SCALE * half * HALF : SCALE * (half + 1) * HALF, :] = (
            res.results[s]["out"]
        )
    return out


def kernel(**inputs) -> np.ndarray:
    return _gather(_run(_host_inputs(**inputs)))



# revision 2
# speedup vs baseline: 1.0139x; 1.0139x over previous
"""CARAFE (content-aware upsample) Trainium2 kernel.

Sharding: 8 cores = batch(4) x H-halves(2). Host slices X with 2-row
zero-padded halos; each core computes its full output shard
[64, 128, 256]; host concatenates. No collectives.

Key algebraic simplification: dilation (2) == scale (2), so the
unfold patch for hi-res pixel (2h+r1, 2w+r2), tap (i,j) is
X[c, h+i-2, w+j-2] -- independent of the subpixel (r1,r2). The whole
CARAFE accumulation runs on the low-res grid:
  out_q[c,h,w] = sum_ij A[(i*5+j)*4+q, h, w] * X[c, h+i-2, w+j-2]
with A = softmax(pixel-shuffled encoder logits).

Softmax normalization is deferred: E = exp(logits) (logits are O(1),
max-subtraction unnecessary), denominators D_q = sum_k E[4k+q] via a
one-hot matmul, and E^T is scaled by 1/D once in pixel-major layout.
"""

import numpy as np

SCALE = 2
KUP = 5
EPS = 1e-5
B, C, H, W = 4, 64, 128, 128
CMID = 64
ENC = 100  # (SCALE*KUP)**2
HALF = H // 2          # 64 low-res rows per core
HL = HALF + 4          # 68 rows of X incl. 2-row halos
WM1R = HALF + 2        # 66 rows of compressed features (1-row halo)
WM1W = W + 2           # 130 cols (1-col zero pad each side)

USE_F32R = False       # fp32 data streamed at 1 cyc/row on PE (N>=256)
GP_TAPS = 0            # GpSimd lacks TensorScalarPtr in the V3 ISA; keep 0
FUSED_NORM = True      # normalize E^T by 1/D in one big rank-4-AP op
REPS = 1               # in-NEFF repetitions (timing only; leave 1 for grading)


def _build_program():
    import concourse.bass as bass
    import concourse.tile as tile
    from concourse import mybir
    from concourse.vector_clock import ScopedClock

    f32 = mybir.dt.float32
    f32r = mybir.dt.float32r
    mm_dt = f32r if USE_F32R else f32

    class SplitDrainTC(tile.TileContext):
        # walrus in this container rejects >2 sync waits on one CTRL
        # instruction; put each tail-drain wait on its own SP nop.
        def _drain_and_barrier(self, tick_clock, wait_clock):
            probe = self.nc.sync.nop()
            wait_clock.add_sem_waits(
                probe.ins, ScopedClock({None: tick_clock.global_clock})
            )
            waits = list(probe.ins.sync_info.on_wait) if probe.ins.sync_info else []
            if probe.ins.sync_info:
                probe.ins.sync_info.on_wait = []
            for w in waits:
                n = self.nc.sync.nop()
                if n.ins.sync_info is None:
                    n.ins.sync_info = mybir.SyncInfo(on_wait=[w], on_update=[])
                else:
                    n.ins.sync_info.on_wait = [w]
            self.nc.sync.drain()
            self.nc.all_engine_barrier()
            assert self.sems is not None
            popped = self.nc._tile_sem_poison_stack.pop()
            assert popped is self._sem_poison
            self.nc.clear_and_free_semaphores(list(self.sems.allocated().values()))
            self.nc.all_engine_barrier()

    nc = bass.Bass()
    ap_in = {}
    for name, shape in [
        ("Xh", [C, HL * W]),
        ("W1", [C, CMID]),
        ("W3", [C, 9 * ENC]),
        ("c1s", [CMID, 1]),
        ("c1b", [CMID, 1]),
        ("c3s", [ENC, 1]),
        ("c3b", [ENC, 1]),
        ("sel", [ENC, 4]),
        ("ident", [128, 128]),
    ]:
        ap_in[name] = nc.dram_tensor(name, shape, f32, kind="ExternalInput").ap()
    out_d = nc.dram_tensor("out", [C, SCALE * HALF, SCALE * W], f32,
                           kind="ExternalOutput").ap()

    mult = mybir.AluOpType.mult
    add = mybir.AluOpType.add
    AF = mybir.ActivationFunctionType

    with SplitDrainTC(nc) as tc:
        for _ in range(REPS):
            _build_tile_kernel(tc, nc, ap_in, out_d, mm_dt, mult, add, AF,
                               bass, mybir)
    _split_sync_waits(nc, mybir)
    return nc


def _split_sync_waits(nc, mybir, max_waits=1):
    """walrus in this container rejects multiple sync waits on some
    instruction structs (Matmult allows just one);
    hoist the excess onto same-engine nops placed just before."""
    ctr = 0
    for bb in nc.m.functions[0].blocks:
        new = []
        changed = False
        for inst in bb.instructions:
            si = inst.sync_info
            waits = list(si.on_wait) if si and si.on_wait else []
            if len(waits) > max_waits:
                extra, keep = waits[:-max_waits], waits[-max_waits:]
                for i in range(0, len(extra), max_waits):
                    ctr += 1
                    nop = mybir.InstNoOp(name=f"wsplit-{ctr}", ins=[], outs=[])
                    nop.engine = inst.engine
                    nop.sync_info = mybir.SyncInfo(
                        on_wait=extra[i : i + max_waits], on_update=[]
                    )
                    new.append(nop)
                si.on_wait = keep
                changed = True
            new.append(inst)
        if changed:
            bb.instructions = new


def _build_tile_kernel(tc, nc, ap_in, out_d, mm_dt, mult, add, AF, bass, mybir):
    f32 = mybir.dt.float32
    ctxs = []

    def pool(name, bufs, space="SBUF"):
        p = tc.tile_pool(name=name, bufs=bufs, space=space)
        ctxs.append(p)
        return p.__enter__()

    consts = pool("consts", 1)
    persist = pool("persist", 1)
    psA = pool("psA", 2, space="PSUM")     # conv matmul outputs
    psT = pool("psT", 2, space="PSUM")     # transposes
    psD = pool("psD", 1, space="PSUM")     # softmax denominators
    accp = pool("acc", 4)
    stagep = pool("stage", 4)

    # ---- constants ----
    W1 = consts.tile([C, CMID], f32, tag="w1")
    nc.sync.dma_start(W1[:], ap_in["W1"][:])
    W3 = consts.tile([C, 9 * ENC], f32, tag="w3")
    nc.sync.dma_start(W3[:], ap_in["W3"][:])
    c1s = consts.tile([CMID, 1], f32, tag="c1s")
    nc.sync.dma_start(c1s[:], ap_in["c1s"][:])
    c1b = consts.tile([CMID, 1], f32, tag="c1b")
    nc.sync.dma_start(c1b[:], ap_in["c1b"][:])
    c3s = consts.tile([ENC, 1], f32, tag="c3s")
    nc.sync.dma_start(c3s[:], ap_in["c3s"][:])
    c3b = consts.tile([ENC, 1], f32, tag="c3b")
    nc.sync.dma_start(c3b[:], ap_in["c3b"][:])
    sel = consts.tile([ENC, 4], f32, tag="sel")
    nc.sync.dma_start(sel[:], ap_in["sel"][:])
    ident = consts.tile([128, 128], f32, tag="ident")
    nc.sync.dma_start(ident[:], ap_in["ident"][:])

    # ---- load X ----
    Xh = persist.tile([C, HL * W], f32, tag="xh")
    nc.sync.dma_start(Xh[:], ap_in["Xh"][:])

    # ---- compress: 1x1 conv + BN + ReLU -> Wm1 [C, 66 x 130] ----
    Wm1 = persist.tile([C, WM1R * WM1W], f32, tag="wm1")
    nc.gpsimd.memset(Wm1.rearrange("p (r w) -> p r w", w=WM1W)[:, :, 0:1], 0.0)
    nc.gpsimd.memset(
        Wm1.rearrange("p (r w) -> p r w", w=WM1W)[:, :, WM1W - 1 : WM1W], 0.0
    )
    r = 0
    while r < WM1R:
        rows = min(4, WM1R - r)
        n = rows * W
        ps = psA.tile([CMID, 512], f32, tag="ps")
        nc.tensor.matmul(
            ps[:, :n],
            W1[:].bitcast(mm_dt),
            Xh[:, (r + 1) * W : (r + 1 + rows) * W].bitcast(mm_dt),
            start=True, stop=True,
        )
        wm_view = Wm1.rearrange("p (r w) -> p r w", w=WM1W)
        nc.scalar.activation(
            wm_view[:, r : r + rows, 1 : 1 + W],
            ps[:, :n].rearrange("p (r w) -> p r w", w=W),
            AF.Relu, bias=c1b[:], scale=c1s[:],
        )
        r += rows

    # ---- X transpose: Xt [128w, (5 j-shifts, 68 rows, 64 c)] ----
    Xt = persist.tile([128, 5 * HL * C], f32, tag="xt")
    xt_v = Xt.rearrange("p (j r c) -> p j r c", j=5, c=C)
    xh_v = Xh.rearrange("p (r w) -> p r w", w=W)
    for rho in range(HL):
        pt = psT.tile([128, 128], f32, tag="ps")
        nc.tensor.transpose(pt[:, :C], xh_v[:, rho, :], ident[:C, :C])
        nc.scalar.copy(xt_v[:, 2, rho, :], pt[:, :C])
    for j in [0, 1, 3, 4]:
        d = j - 2
        p0, p1 = max(0, -d), 128 - max(0, d)
        # zero the whole band first (gpsimd memset needs aligned start
        # partitions); the shift DMA then overwrites the interior.
        nc.gpsimd.memset(xt_v[:, j, :, :], 0.0)
        nc.sync.dma_start(
            out=xt_v[p0:p1, j, :, :], in_=xt_v[p0 + d : p1 + d, 2, :, :]
        )

    # ---- encode: 3x3 conv + affine + exp -> E [100, 64*128] ----
    # E reuses Xh's SBUF slot (Xh is fully consumed by conv1 + transposes).
    E = persist.tile([ENC, HALF * W], f32, tag="xh")
    wm_flat = Wm1[:]
    for t0 in range(0, HALF, 4):
        ps = psA.tile([ENC, 512], f32, tag="ps")
        for ti, (di, dj) in enumerate((di, dj) for di in range(3) for dj in range(3)):
            off = (t0 + di) * WM1W + dj
            mv = bass.AP(
                tensor=wm_flat.tensor,
                offset=wm_flat.offset + off,
                ap=[wm_flat.ap[0], [WM1W, 4], [1, W]],
            )
            nc.tensor.matmul(
                ps[:],
                W3[:, ti * ENC : (ti + 1) * ENC].bitcast(mm_dt),
                mv.bitcast(mm_dt),
                start=(ti == 0), stop=(ti == 8),
            )
        nc.scalar.activation(
            E[:, t0 * W : (t0 + 4) * W], ps[:], AF.Exp, bias=c3b[:], scale=c3s[:]
        )

    # ---- softmax denominators: D^T then 1/D ----
    pd = psD.tile([128, 4 * HALF], f32, tag="pd")
    for t in range(HALF):
        nc.tensor.matmul(
            pd[:, t * 4 : (t + 1) * 4],
            E[:, t * W : (t + 1) * W].bitcast(mm_dt),
            sel[:].bitcast(mm_dt),
            start=True, stop=True,
        )
    Rt = persist.tile([128, 4 * HALF], f32, tag="rt")
    nc.vector.reciprocal(Rt[:], pd[:])

    # ---- E^T (pixel-major) and fused normalization ----
    # Et reuses Wm1's slot (Wm1 fully consumed by the 3x3 conv).
    Et = persist.tile([128, HALF * ENC], f32, tag="wm1")
    et_v = Et.rearrange("p (t k) -> p t k", k=ENC)
    for t in range(HALF):
        pt = psT.tile([128, 128], f32, tag="ps")
        nc.tensor.transpose(pt[:, :ENC], E[:, t * W : (t + 1) * W], ident[:ENC, :ENC])
        nc.scalar.copy(et_v[:, t, :], pt[:, :ENC])
    if FUSED_NORM:
        bcast = bass.AP(
            tensor=Rt.tensor, offset=Rt.offset,
            ap=[Rt.ap[0], [4, HALF], [0, 25], [1, 4]],
        )
        nc.vector.tensor_tensor(Et[:], Et[:], bcast, mult)
    else:
        for t in range(HALF):
            bc = bass.AP(
                tensor=Rt.tensor, offset=Rt.offset + t * 4,
                ap=[Rt.ap[0], [0, 25], [1, 4]],
            )
            nc.vector.tensor_tensor(et_v[:, t, :], et_v[:, t, :], bc, mult)

    # ---- CARAFE accumulation + pixel-shuffle writeout ----
    # Per (t, q): ONE big tensor_tensor product op over all 25 taps
    # (free dims (c, j, i); X via strided AP into Xt's 5 j-shifted
    # copies, coefficients broadcast over c with a stride-0 dim), then
    # ONE reduce_sum over the 25 contiguous taps. Replaces 25 tiny
    # scalar_tensor_tensor ops whose ~220-cycle fixed overhead
    # dominated the 64-elem payload.
    prodp = pool("prod", 3)
    for t in range(HALF):
        stage = stagep.tile([C, 2 * 2 * W], f32, tag="stage")
        for q in range(4):
            r1, r2 = q // 2, q % 2
            x_ap = bass.AP(
                tensor=Xt.tensor, offset=Xt.offset + t * C,
                ap=[Xt.ap[0], [1, C], [HL * C, 5], [C, 5]],
            )
            a_ap = bass.AP(
                tensor=Et.tensor, offset=Et.offset + t * ENC + q,
                ap=[Et.ap[0], [0, C], [4, 5], [20, 5]],
            )
            P = prodp.tile([128, C * 25], f32, tag="prod")
            p_ap = bass.AP(
                tensor=P.tensor, offset=P.offset,
                ap=[P.ap[0], [25, C], [5, 5], [1, 5]],
            )
            nc.vector.tensor_tensor(p_ap, x_ap, a_ap, mult)
            acc = accp.tile([128, C], f32, tag="acc")
            red_in = bass.AP(
                tensor=P.tensor, offset=P.offset,
                ap=[P.ap[0], [25, C], [1, 25]],
            )
            nc.vector.reduce_sum(acc[:], red_in, axis=mybir.AxisListType.X)
            po = psT.tile([128, 128], f32, tag="ps")
            nc.tensor.transpose(po[:C, :], acc[:], ident[:, :])
            st_v = stage.rearrange("p (r x) -> p r x", r=2)
            out_ap = bass.AP(
                tensor=st_v.tensor, offset=st_v.offset + r1 * 2 * W + r2,
                ap=[st_v.ap[0], [2, W]],
            )
            nc.scalar.copy(out_ap, po[:C, :])
        nc.sync.dma_start(
            out_d[:, 2 * t : 2 * t + 2, :], stage.rearrange("p (r x) -> p r x", r=2)
        )

    for p in reversed(ctxs):
        p.__exit__(None, None, None)


def _host_inputs(X, comp_w, comp_gamma, comp_beta, comp_mean, comp_var,
                 enc_w, enc_b, enc_gamma, enc_beta, enc_mean, enc_var):
    X = np.asarray(X, np.float32)
    inv1 = (np.asarray(comp_gamma, np.float32)
            / np.sqrt(np.asarray(comp_var, np.float32) + EPS))
    b1 = np.asarray(comp_beta, np.float32) - np.asarray(comp_mean, np.float32) * inv1
    inv3 = (np.asarray(enc_gamma, np.float32)
            / np.sqrt(np.asarray(enc_var, np.float32) + EPS))
    b3 = ((np.asarray(enc_b, np.float32) - np.asarray(enc_mean, np.float32)) * inv3
          + np.asarray(enc_beta, np.float32))

    W1 = np.ascontiguousarray(np.asarray(comp_w, np.float32)[:, :, 0, 0].T)
    # W3[c_in, tap*100 + c_out]
    W3 = np.ascontiguousarray(
        np.asarray(enc_w, np.float32).transpose(2, 3, 1, 0).reshape(9 * C, ENC)
        .reshape(9, C, ENC).transpose(1, 0, 2).reshape(C, 9 * ENC)
    )
    sel = np.zeros((ENC, 4), np.float32)
    sel[np.arange(ENC), np.arange(ENC) % 4] = 1.0
    ident = np.eye(128, dtype=np.float32)

    common = dict(
        W1=W1, W3=W3,
        c1s=inv1.reshape(CMID, 1), c1b=b1.reshape(CMID, 1),
        c3s=inv3.reshape(ENC, 1), c3b=b3.reshape(ENC, 1),
        sel=sel, ident=ident,
    )
    in_maps = []
    for s in range(8):
        b, half = divmod(s, 2)
        h0 = half * HALF
        xs = np.zeros((C, HL, W), np.float32)
        lo, hi = h0 - 2, h0 + HALF + 2
        clo, chi = max(lo, 0), min(hi, H)
        xs[:, clo - lo : clo - lo + (chi - clo), :] = X[b, :, clo:chi, :]
        in_maps.append(dict(Xh=xs.reshape(C, HL * W), **common))
    return in_maps


_PROGRAM_CACHE = {}


def _run(in_maps, trace=False, **kw):
    from concourse.bass_utils import run_bass_kernel_spmd

    if "nc" not in _PROGRAM_CACHE:
        _PROGRAM_CACHE["nc"] = _build_program()
    nc = _PROGRAM_CACHE["nc"]
    return run_bass_kernel_spmd(nc, in_maps, list(range(8)), trace=trace, **kw)


def _gather(res):
    out = np.zeros((B, C, SCALE * H, SCALE * W), np.float32)
    for s in range(8):
        b, half = divmod(s, 2)
        out[b, :, SCALE * half * HALF : SCALE * (half + 1) * HALF, :] = (
            res.results[s]["out"]
        )
    return out


def kernel(**inputs) -> np.ndarray:
    return _gather(_run(_host_inputs(**inputs)))



# revision 5
# speedup vs baseline: 1.0513x; 1.0369x over previous
"""CARAFE (content-aware upsample) Trainium2 kernel.

Sharding: 8 cores = batch(4) x H-halves(2). Host slices X with 2-row
zero-padded halos; each core computes its full output shard
[64, 128, 256]; host concatenates. No collectives.

Key algebraic simplification: dilation (2) == scale (2), so the
unfold patch for hi-res pixel (2h+r1, 2w+r2), tap (i,j) is
X[c, h+i-2, w+j-2] -- independent of the subpixel (r1,r2). The whole
CARAFE accumulation runs on the low-res grid:
  out_q[c,h,w] = sum_ij A[(i*5+j)*4+q, h, w] * X[c, h+i-2, w+j-2]
with A = softmax(pixel-shuffled encoder logits).

Softmax normalization is deferred: E = exp(logits) (logits are O(1),
max-subtraction unnecessary), denominators D_q = sum_k E[4k+q] via a
one-hot matmul, and E^T is scaled by 1/D once in pixel-major layout.
"""

import numpy as np

SCALE = 2
KUP = 5
EPS = 1e-5
B, C, H, W = 4, 64, 128, 128
CMID = 64
ENC = 100  # (SCALE*KUP)**2
HALF = H // 2          # 64 low-res rows per core
HL = HALF + 4          # 68 rows of X incl. 2-row halos
WM1R = HALF + 2        # 66 rows of compressed features (1-row halo)
WM1W = W + 2           # 130 cols (1-col zero pad each side)

USE_F32R = False       # fp32 data streamed at 1 cyc/row on PE (N>=256)
GP_TAPS = 0            # GpSimd lacks TensorScalarPtr in the V3 ISA; keep 0
FUSED_NORM = True      # normalize E^T by 1/D in one big rank-4-AP op
REPS = 1               # in-NEFF repetitions (timing only; leave 1 for grading)


def _build_program():
    import concourse.bass as bass
    import concourse.tile as tile
    from concourse import mybir
    from concourse.vector_clock import ScopedClock

    f32 = mybir.dt.float32
    f32r = mybir.dt.float32r
    mm_dt = f32r if USE_F32R else f32

    class SplitDrainTC(tile.TileContext):
        # walrus in this container rejects >2 sync waits on one CTRL
        # instruction; put each tail-drain wait on its own SP nop.
        def _drain_and_barrier(self, tick_clock, wait_clock):
            probe = self.nc.sync.nop()
            wait_clock.add_sem_waits(
                probe.ins, ScopedClock({None: tick_clock.global_clock})
            )
            waits = list(probe.ins.sync_info.on_wait) if probe.ins.sync_info else []
            if probe.ins.sync_info:
                probe.ins.sync_info.on_wait = []
            for w in waits:
                n = self.nc.sync.nop()
                if n.ins.sync_info is None:
                    n.ins.sync_info = mybir.SyncInfo(on_wait=[w], on_update=[])
                else:
                    n.ins.sync_info.on_wait = [w]
            self.nc.sync.drain()
            self.nc.all_engine_barrier()
            assert self.sems is not None
            popped = self.nc._tile_sem_poison_stack.pop()
            assert popped is self._sem_poison
            self.nc.clear_and_free_semaphores(list(self.sems.allocated().values()))
            self.nc.all_engine_barrier()

    nc = bass.Bass()
    ap_in = {}
    for name, shape in [
        ("Xh", [C, HL * W]),
        ("W1", [C, CMID]),
        ("W3", [C, 9 * ENC]),
        ("c1s", [CMID, 1]),
        ("c1b", [CMID, 1]),
        ("c3s", [ENC, 1]),
        ("c3b", [ENC, 1]),
        ("sel", [ENC, 4]),
        ("ident", [128, 128]),
    ]:
        ap_in[name] = nc.dram_tensor(name, shape, f32, kind="ExternalInput").ap()
    out_d = nc.dram_tensor("out", [C, SCALE * HALF, SCALE * W], f32,
                           kind="ExternalOutput").ap()

    mult = mybir.AluOpType.mult
    add = mybir.AluOpType.add
    AF = mybir.ActivationFunctionType

    with SplitDrainTC(nc) as tc:
        for _ in range(REPS):
            _build_tile_kernel(tc, nc, ap_in, out_d, mm_dt, mult, add, AF,
                               bass, mybir)
    _split_sync_waits(nc, mybir)
    return nc


def _split_sync_waits(nc, mybir, max_waits=1):
    """walrus in this container rejects multiple sync waits on some
    instruction structs (Matmult allows just one);
    hoist the excess onto same-engine nops placed just before."""
    ctr = 0
    for bb in nc.m.functions[0].blocks:
        new = []
        changed = False
        for inst in bb.instructions:
            si = inst.sync_info
            waits = list(si.on_wait) if si and si.on_wait else []
            if len(waits) > max_waits:
                extra, keep = waits[:-max_waits], waits[-max_waits:]
                for i in range(0, len(extra), max_waits):
                    ctr += 1
                    nop = mybir.InstNoOp(name=f"wsplit-{ctr}", ins=[], outs=[])
                    nop.engine = inst.engine
                    nop.sync_info = mybir.SyncInfo(
                        on_wait=extra[i : i + max_waits], on_update=[]
                    )
                    new.append(nop)
                si.on_wait = keep
                changed = True
            new.append(inst)
        if changed:
            bb.instructions = new


def _build_tile_kernel(tc, nc, ap_in, out_d, mm_dt, mult, add, AF, bass, mybir):
    f32 = mybir.dt.float32
    ctxs = []

    def pool(name, bufs, space="SBUF"):
        p = tc.tile_pool(name=name, bufs=bufs, space=space)
        ctxs.append(p)
        return p.__enter__()

    consts = pool("consts", 1)
    persist = pool("persist", 1)
    psA = pool("psA", 2, space="PSUM")     # conv matmul outputs
    psT = pool("psT", 2, space="PSUM")     # transposes
    psD = pool("psD", 1, space="PSUM")     # softmax denominators
    accp = pool("acc", 4)
    stagep = pool("stage", 4)

    # ---- constants ----
    W1 = consts.tile([C, CMID], f32, tag="w1")
    nc.sync.dma_start(W1[:], ap_in["W1"][:])
    W3 = consts.tile([C, 9 * ENC], f32, tag="w3")
    nc.sync.dma_start(W3[:], ap_in["W3"][:])
    c1s = consts.tile([CMID, 1], f32, tag="c1s")
    nc.sync.dma_start(c1s[:], ap_in["c1s"][:])
    c1b = consts.tile([CMID, 1], f32, tag="c1b")
    nc.sync.dma_start(c1b[:], ap_in["c1b"][:])
    c3s = consts.tile([ENC, 1], f32, tag="c3s")
    nc.sync.dma_start(c3s[:], ap_in["c3s"][:])
    c3b = consts.tile([ENC, 1], f32, tag="c3b")
    nc.sync.dma_start(c3b[:], ap_in["c3b"][:])
    sel = consts.tile([ENC, 4], f32, tag="sel")
    nc.sync.dma_start(sel[:], ap_in["sel"][:])
    ident = consts.tile([128, 128], f32, tag="ident")
    nc.sync.dma_start(ident[:], ap_in["ident"][:])

    # ---- load X ----
    Xh = persist.tile([C, HL * W], f32, tag="xh")
    nc.sync.dma_start(Xh[:], ap_in["Xh"][:])

    # ---- compress: 1x1 conv + BN + ReLU -> Wm1 [C, 66 x 130] ----
    Wm1 = persist.tile([C, WM1R * WM1W], f32, tag="wm1")
    nc.gpsimd.memset(Wm1.rearrange("p (r w) -> p r w", w=WM1W)[:, :, 0:1], 0.0)
    nc.gpsimd.memset(
        Wm1.rearrange("p (r w) -> p r w", w=WM1W)[:, :, WM1W - 1 : WM1W], 0.0
    )
    r = 0
    while r < WM1R:
        rows = min(4, WM1R - r)
        n = rows * W
        ps = psA.tile([CMID, 512], f32, tag="ps")
        nc.tensor.matmul(
            ps[:, :n],
            W1[:].bitcast(mm_dt),
            Xh[:, (r + 1) * W : (r + 1 + rows) * W].bitcast(mm_dt),
            start=True, stop=True,
        )
        wm_view = Wm1.rearrange("p (r w) -> p r w", w=WM1W)
        nc.scalar.activation(
            wm_view[:, r : r + rows, 1 : 1 + W],
            ps[:, :n].rearrange("p (r w) -> p r w", w=W),
            AF.Relu, bias=c1b[:], scale=c1s[:],
        )
        r += rows

    # ---- X transpose: Xt [128w, (5 j-shifts, 68 rows, 64 c)] ----
    Xt = persist.tile([128, 5 * HL * C], f32, tag="xt")
    xt_v = Xt.rearrange("p (j r c) -> p j r c", j=5, c=C)
    xh_v = Xh.rearrange("p (r w) -> p r w", w=W)
    for rho in range(HL):
        pt = psT.tile([128, 128], f32, tag="ps")
        nc.tensor.transpose(pt[:, :C], xh_v[:, rho, :], ident[:C, :C])
        nc.scalar.copy(xt_v[:, 2, rho, :], pt[:, :C])
    for j in [0, 1, 3, 4]:
        d = j - 2
        p0, p1 = max(0, -d), 128 - max(0, d)
        # zero the whole band first (gpsimd memset needs aligned start
        # partitions); the shift DMA then overwrites the interior.
        nc.gpsimd.memset(xt_v[:, j, :, :], 0.0)
        nc.sync.dma_start(
            out=xt_v[p0:p1, j, :, :], in_=xt_v[p0 + d : p1 + d, 2, :, :]
        )

    # ---- encode: 3x3 conv + affine + exp -> E [100, 64*128] ----
    # E reuses Xh's SBUF slot (Xh is fully consumed by conv1 + transposes).
    E = persist.tile([ENC, HALF * W], f32, tag="xh")
    wm_flat = Wm1[:]
    for t0 in range(0, HALF, 4):
        ps = psA.tile([ENC, 512], f32, tag="ps")
        for ti, (di, dj) in enumerate((di, dj) for di in range(3) for dj in range(3)):
            off = (t0 + di) * WM1W + dj
            mv = bass.AP(
                tensor=wm_flat.tensor,
                offset=wm_flat.offset + off,
                ap=[wm_flat.ap[0], [WM1W, 4], [1, W]],
            )
            nc.tensor.matmul(
                ps[:],
                W3[:, ti * ENC : (ti + 1) * ENC].bitcast(mm_dt),
                mv.bitcast(mm_dt),
                start=(ti == 0), stop=(ti == 8),
            )
        nc.scalar.activation(
            E[:, t0 * W : (t0 + 4) * W], ps[:], AF.Exp, bias=c3b[:], scale=c3s[:]
        )

    # ---- softmax denominators: D^T then 1/D ----
    pd = psD.tile([128, 4 * HALF], f32, tag="pd")
    for t in range(HALF):
        nc.tensor.matmul(
            pd[:, t * 4 : (t + 1) * 4],
            E[:, t * W : (t + 1) * W].bitcast(mm_dt),
            sel[:].bitcast(mm_dt),
            start=True, stop=True,
        )
    Rt = persist.tile([128, 4 * HALF], f32, tag="rt")
    nc.vector.reciprocal(Rt[:], pd[:])

    # ---- E^T (pixel-major) and fused normalization ----
    # Et reuses Wm1's slot (Wm1 fully consumed by the 3x3 conv).
    Et = persist.tile([128, HALF * ENC], f32, tag="wm1")
    et_v = Et.rearrange("p (t k) -> p t k", k=ENC)
    for t in range(HALF):
        pt = psT.tile([128, 128], f32, tag="ps")
        nc.tensor.transpose(pt[:, :ENC], E[:, t * W : (t + 1) * W], ident[:ENC, :ENC])
        nc.scalar.copy(et_v[:, t, :], pt[:, :ENC])
    if FUSED_NORM:
        bcast = bass.AP(
            tensor=Rt.tensor, offset=Rt.offset,
            ap=[Rt.ap[0], [4, HALF], [0, 25], [1, 4]],
        )
        nc.vector.tensor_tensor(Et[:], Et[:], bcast, mult)
    else:
        for t in range(HALF):
            bc = bass.AP(
                tensor=Rt.tensor, offset=Rt.offset + t * 4,
                ap=[Rt.ap[0], [0, 25], [1, 4]],
            )
            nc.vector.tensor_tensor(et_v[:, t, :], et_v[:, t, :], bc, mult)

    # ---- CARAFE accumulation + pixel-shuffle writeout ----
    # Per (t, q): ONE big tensor_tensor product op over all 25 taps
    # (free dims (c, j, i); X via strided AP into Xt's 5 j-shifted
    # copies, coefficients broadcast over c with a stride-0 dim), then
    # ONE reduce_sum over the 25 contiguous taps. Replaces 25 tiny
    # scalar_tensor_tensor ops whose ~220-cycle fixed overhead
    # dominated the 64-elem payload.
    prodp = pool("prod", 3)
    for t in range(HALF):
        stage = stagep.tile([C, 2 * 2 * W], f32, tag="stage")
        for q in range(4):
            r1, r2 = q // 2, q % 2
            # iteration order (j, i, c): innermost runs of 64 on every
            # port (strided/zero strides are fine; short 5-elem runs
            # cost ~2x in AP-walker rollovers)
            x_ap = bass.AP(
                tensor=Xt.tensor, offset=Xt.offset + t * C,
                ap=[Xt.ap[0], [HL * C, 5], [C, 5], [1, C]],
            )
            a_ap = bass.AP(
                tensor=Et.tensor, offset=Et.offset + t * ENC + q,
                ap=[Et.ap[0], [4, 5], [20, 5], [0, C]],
            )
            P = prodp.tile([128, C * 25], f32, tag="prod")
            p_ap = bass.AP(
                tensor=P.tensor, offset=P.offset,
                ap=[P.ap[0], [5, 5], [1, 5], [25, C]],
            )
            nc.vector.tensor_tensor(p_ap, x_ap, a_ap, mult)
            acc = accp.tile([128, C], f32, tag="acc")
            red_in = bass.AP(
                tensor=P.tensor, offset=P.offset,
                ap=[P.ap[0], [25, C], [1, 25]],
            )
            nc.vector.reduce_sum(acc[:], red_in, axis=mybir.AxisListType.X)
            po = psT.tile([128, 128], f32, tag="ps")
            nc.tensor.transpose(po[:C, :], acc[:], ident[:, :])
            st_v = stage.rearrange("p (r x) -> p r x", r=2)
            out_ap = bass.AP(
                tensor=st_v.tensor, offset=st_v.offset + r1 * 2 * W + r2,
                ap=[st_v.ap[0], [2, W]],
            )
            nc.scalar.copy(out_ap, po[:C, :])
        nc.sync.dma_start(
            out_d[:, 2 * t : 2 * t + 2, :], stage.rearrange("p (r x) -> p r x", r=2)
        )

    for p in reversed(ctxs):
        p.__exit__(None, None, None)


def _host_inputs(X, comp_w, comp_gamma, comp_beta, comp_mean, comp_var,
                 enc_w, enc_b, enc_gamma, enc_beta, enc_mean, enc_var):
    X = np.asarray(X, np.float32)
    inv1 = (np.asarray(comp_gamma, np.float32)
            / np.sqrt(np.asarray(comp_var, np.float32) + EPS))
    b1 = np.asarray(comp_beta, np.float32) - np.asarray(comp_mean, np.float32) * inv1
    inv3 = (np.asarray(enc_gamma, np.float32)
            / np.sqrt(np.asarray(enc_var, np.float32) + EPS))
    b3 = ((np.asarray(enc_b, np.float32) - np.asarray(enc_mean, np.float32)) * inv3
          + np.asarray(enc_beta, np.float32))

    W1 = np.ascontiguousarray(np.asarray(comp_w, np.float32)[:, :, 0, 0].T)
    # W3[c_in, tap*100 + c_out]
    W3 = np.ascontiguousarray(
        np.asarray(enc_w, np.float32).transpose(2, 3, 1, 0).reshape(9 * C, ENC)
        .reshape(9, C, ENC).transpose(1, 0, 2).reshape(C, 9 * ENC)
    )
    sel = np.zeros((ENC, 4), np.float32)
    sel[np.arange(ENC), np.arange(ENC) % 4] = 1.0
    ident = np.eye(128, dtype=np.float32)

    common = dict(
        W1=W1, W3=W3,
        c1s=inv1.reshape(CMID, 1), c1b=b1.reshape(CMID, 1),
        c3s=inv3.reshape(ENC, 1), c3b=b3.reshape(ENC, 1),
        sel=sel, ident=ident,
    )
    in_maps = []
    for s in range(8):
        b, half = divmod(s, 2)
        h0 = half * HALF
        xs = np.zeros((C, HL, W), np.float32)
        lo, hi = h0 - 2, h0 + HALF + 2
        clo, chi = max(lo, 0), min(hi, H)
        xs[:, clo - lo : clo - lo + (chi - clo), :] = X[b, :, clo:chi, :]
        in_maps.append(dict(Xh=xs.reshape(C, HL * W), **common))
    return in_maps


_PROGRAM_CACHE = {}


def _run(in_maps, trace=False, **kw):
    from concourse.bass_utils import run_bass_kernel_spmd

    if "nc" not in _PROGRAM_CACHE:
        _PROGRAM_CACHE["nc"] = _build_program()
    nc = _PROGRAM_CACHE["nc"]
    return run_bass_kernel_spmd(nc, in_maps, list(range(8)), trace=trace, **kw)


def _gather(res):
    out = np.zeros((B, C, SCALE * H, SCALE * W), np.float32)
    for s in range(8):
        b, half = divmod(s, 2)
        out[b, :, SCALE * half * HALF : SCALE * (half + 1) * HALF, :] = (
            res.results[s]["out"]
        )
    return out


def kernel(**inputs) -> np.ndarray:
    return _gather(_run(_host_inputs(**inputs)))

